# revision 14
# baseline (speedup 1.0000x reference)
"""KAN-SE (squeeze-excite with 2-layer KAN MLP) Trainium2 kernel.

Full-input contract: kernel(**inputs) takes the complete (32, 512, 64, 64)
batch plus KAN weights, shards the batch across 8 NeuronCores (4 samples
per core, data-parallel, weights replicated), and returns the full output.

The rel-err gate is 2e-2 (fp32 pipeline measured 4e-7), so precision is
traded for bandwidth/throughput (verified ~4e-4 l2 end to end): x/y move
over HBM as fp16 (host casts both ways), KAN weights/features are fp16 on
the PE, sums/activations stay f32.

v5 pipeline shape (driven by per-instruction traces of v2..v4):
  - const DMAs first (v4 emitted them late and sample-0's KAN stalled
    12us on weights), then all 16 tile-load doorbells, so load transfers
    stream back-to-back at the ~330 GB/s per-core read rate.
  - each load is followed by its row-sum on a rotating engine (DVE
    tensor_scalar+accum / ScalarE Copy+accum, both in-place; the reduce
    path gets no fp16 speedup so it must not serialize on one engine).
    The last tile's sum is a 2-stage DVE op (fp16 4x add of halves, then
    a half-length reduce) to shorten the tail.
  - KAN for samples {0,1} runs pair-batched (2-col rhs); samples 2 and 3
    run alone so their gates chase the load stream.  Layer-1 b-splines
    are emitted groups-0..2 batched + group-3 alone: only ~9 matmuls and
    a small emit depend on a sample's final tile.
  - layer-2 runs per out-group: each 9-matmul chain immediately feeds
    sigmoid -> gate-scale -> store doorbell, so stores dribble out
    og-by-og instead of waiting for the full 512-wide gate.
  - ScalarE only ever runs Sigmoid/Copy (SiLU = x*sigmoid(x), mult on
    DVE); both act tables are preloaded.  Gate scales are mostly DVE
    (fp16 hits the 4x DVE mode, ~1.3us/tile).

Per-core HBM traffic: 16 MiB in + 16 MiB out (fp16), read-once/write-once.
"""

import numpy as np

# ---- problem constants (hardcoded per contract; do not read spec/reference) ----
B, C, H, W = 32, 512, 64, 64
HIDDEN = 64            # max(16, 512 // 8)
KB = 8                 # GRID_SIZE + SPLINE_ORDER = 5 + 3
NCORES = 8
NS = B // NCORES       # samples per core = 4
NG = C // 128          # channel groups of 128 = 4
HWPIX = H * W          # 4096
NF = KB + 1            # features per channel: silu + 8 spline bases

# row-sum engine per tile index t = n*4+g (V=DVE, S=ScalarE, 2=DVE 2-stage);
# ScalarE takes most (3.7us vs DVE 4.4us, and DVE owns the b-splines/scales)
SUM_ENG = "SVSV" "SVSV" "SVSV" "SVS2"
# gate-multiply engine per tile index (DVE 4x fp16 is ~3x faster than ScalarE)
SCALE_ENG = "VSVV" "VVSV" "VVVV" "VVVV"


def _grid_cols(grid_row: np.ndarray, xscale: float, nsg: int):
    """Packed per-group-replicated grid constant columns for the batched
    Cox-de-Boor recurrence, evaluated on inputs x' = x * xscale.

    offsets maps:
      'ge'   -> start of g_i * xscale,        width nsg*12
      (k,'a')-> start of -g_i / (k h),        width nsg*(11-k)
      (k,'c')-> start of  g_{i+k+1} / (k h),  width nsg*(11-k)
      'rs'   -> start of 1/(k h xscale), k=1..3
    """
    g = np.asarray(grid_row, np.float64)
    assert g.shape == (12,)
    h = g[1] - g[0]
    segs, offsets = [], {}
    pos = 0

    def add(key, vals):
        nonlocal pos
        offsets[key] = pos
        segs.append(vals.astype(np.float32))
        pos += vals.size

    add('ge', np.tile(g * xscale, nsg))
    for k in (1, 2, 3):
        w = 11 - k
        add((k, 'a'), np.tile(-g[:w] / (k * h), nsg))
        add((k, 'c'), np.tile(g[k + 1:12] / (k * h), nsg))
    add('rs', np.array([1.0 / (k * h * xscale) for k in (1, 2, 3)]))
    return np.concatenate(segs), offsets


def _host_prep(inputs):
    """Rearrange weights into the SBUF layouts the device program uses."""
    f32, f16 = np.float32, np.float16
    base_w1 = np.asarray(inputs["base_w1"], f32)      # (64, 512)
    spline_w1 = np.asarray(inputs["spline_w1"], f32)  # (64, 512, 8)
    scaler1 = np.asarray(inputs["scaler1"], f32)      # (64, 512)
    base_w2 = np.asarray(inputs["base_w2"], f32)      # (512, 64)
    spline_w2 = np.asarray(inputs["spline_w2"], f32)  # (512, 64, 8)
    scaler2 = np.asarray(inputs["scaler2"], f32)      # (512, 64)

    # layer-1 silu feature arrives as sum*sigmoid(sum/HW) = HW*silu(mean),
    # so fold 1/HW into the base weights.
    # w1t[p, g*64+o] = base_w1[o, 128g+p] / HWPIX
    w1t = (base_w1 / HWPIX).reshape(HIDDEN, NG, 128)
    w1t = w1t.transpose(2, 1, 0).reshape(128, NG * HIDDEN)
    # sw1[p, (g*8+k)*64+o] = (spline_w1*scaler1)[o, 128g+p, k]
    sw1 = (spline_w1 * scaler1[:, :, None]).reshape(HIDDEN, NG, 128, KB)
    sw1 = sw1.transpose(2, 1, 3, 0).reshape(128, NG * KB * HIDDEN)
    # w2t[p, o] = base_w2[o, p]
    w2t = base_w2.T
    # sw2[p, k*512+o] = (spline_w2*scaler2)[o, p, k]
    sw2 = (spline_w2 * scaler2[:, :, None]).transpose(1, 2, 0).reshape(HIDDEN, KB * C)

    # packed grid-constant table: layer1 (on raw sums, xscale=HW, replicated
    # over the 4 groups) then layer2 (xscale=1, single copy)
    c1, off1 = _grid_cols(np.asarray(inputs["grid1"], f32)[0], float(HWPIX), NG)
    c2, off2 = _grid_cols(np.asarray(inputs["grid2"], f32)[0], 1.0, 1)
    off2 = {k: v + c1.size for k, v in off2.items()}
    gtab = np.concatenate([c1, c2])
    gtab_full = np.ascontiguousarray(np.tile(gtab[None, :], (128, 1)))

    tensors = {
        "w1t": np.ascontiguousarray(w1t, f16),
        "sw1": np.ascontiguousarray(sw1, f16),
        "w2t": np.ascontiguousarray(w2t, f16),
        "sw2": np.ascontiguousarray(sw2, f16),
        "gtab": gtab_full,
    }
    return tensors, off1, off2, gtab.size


def _emit_bsplines(nc, mybir, pool, gtab_sb, off, sT3, out_j, P, S, G, g0=0):
    """Cubic B-spline bases for S*G per-partition scalars at once.

    sT3:   AP [P, S, G] of the (pre-scaled) inputs.
    out_j: AP [P, S, G, 8] (may be strided, fp16) for the final bases.
    g0:    first group index (selects the replicated grid-constant cols).
    Grid constants broadcast over S (stride-0); x broadcasts over the basis
    index, so each Cox-de-Boor level is one DVE op over ~S*G*11 elems.
    """
    f32 = mybir.dt.float32
    Alu = mybir.AluOpType

    def rep(key, w):
        o = off[key] + g0 * w
        return gtab_sb[:P, o:o + G * w].rearrange(
            "p (g i) -> p () g i", g=G).broadcast_to([P, S, G, w])

    ge = pool.tile([128, S, G, 12], f32, tag=f"ge{P}{S}{G}", bufs=2)
    xb = sT3.rearrange("p s g -> p s g ()")
    nc.vector.tensor_tensor(
        ge[:P], rep('ge', 12), xb.broadcast_to([P, S, G, 12]), Alu.is_le)
    bprev = pool.tile([128, S, G, 11], f32, tag=f"b0{P}{S}{G}", bufs=2)
    nc.vector.tensor_tensor(
        bprev[:P], ge[:P, :, :, 0:11], ge[:P, :, :, 1:12], Alu.subtract)
    # xr[p, k, s, g] = x * 1/(k h xscale)
    xr = pool.tile([128, 3, S, G], f32, tag=f"xr{P}{S}{G}", bufs=2)
    o = off['rs']
    rs_ap = gtab_sb[:P, o:o + 3].rearrange("p k -> p k () ()")
    nc.vector.tensor_tensor(
        xr[:P], rs_ap.broadcast_to([P, 3, S, G]),
        sT3.rearrange("p s g -> p () s g").broadcast_to([P, 3, S, G]), Alu.mult)
    for k in (1, 2, 3):
        w = 11 - k
        xk = xr[:P, k - 1].rearrange("p s g -> p s g ()").broadcast_to([P, S, G, w])
        a_t = pool.tile([128, S, G, 10], f32, tag=f"bsA{P}{S}{G}", bufs=2)
        c_t = pool.tile([128, S, G, 10], f32, tag=f"bsC{P}{S}{G}", bufs=2)
        # A = (x - g_i)/(k h) = xr + (-g_i/(k h));  C = g_{i+k+1}/(k h) - xr
        nc.vector.tensor_tensor(a_t[:P, :, :, :w], rep((k, 'a'), w), xk, Alu.add)
        nc.vector.tensor_tensor(c_t[:P, :, :, :w], rep((k, 'c'), w), xk, Alu.subtract)
        if k < 3:
            bnext = pool.tile([128, S, G, 10], f32, tag=f"bn{P}{S}{G}", bufs=2)
            outp = bnext[:P, :, :, :w]
        else:
            outp = out_j
        nc.vector.tensor_tensor(
            c_t[:P, :, :, :w], c_t[:P, :, :, :w], bprev[:P, :, :, 1:w + 1], Alu.mult)
        nc.vector.tensor_tensor(outp, a_t[:P, :, :, :w], bprev[:P, :, :, 0:w], Alu.mult)
        nc.vector.tensor_tensor(outp, outp, c_t[:P, :, :, :w], Alu.add)
        if k < 3:
            bprev = bnext


def _build_nc(off1, off2, gtab_cols):
    import concourse.bacc as bacc
    import concourse.bass as bass  # noqa: F401
    import concourse.mybir as mybir
    from concourse.tile import TileContext

    f32 = mybir.dt.float32
    f16 = mybir.dt.float16
    Alu = mybir.AluOpType
    Act = mybir.ActivationFunctionType

    # Bacc (not plain Bass): its compile() runs move_matmul_waits_to_ldweights
    # + generate_event_semaphores, which split multi-waits down to the 1-wait-
    # per-instruction TRN2 ISA limit that walrus enforces.
    nc = bacc.Bacc("TRN2", target_bir_lowering=False)
    x_d = nc.declare_dram_parameter("x", [NS, C, H, W], f16, isOutput=False)
    w1t_d = nc.declare_dram_parameter("w1t", [128, NG * HIDDEN], f16, isOutput=False)
    sw1_d = nc.declare_dram_parameter("sw1", [128, NG * KB * HIDDEN], f16, isOutput=False)
    w2t_d = nc.declare_dram_parameter("w2t", [HIDDEN, C], f16, isOutput=False)
    sw2_d = nc.declare_dram_parameter("sw2", [HIDDEN, KB * C], f16, isOutput=False)
    gtab_d = nc.declare_dram_parameter("gtab", [128, gtab_cols], f32, isOutput=False)
    y_d = nc.declare_dram_parameter("y", [NS, C, H, W], f16, isOutput=True)

    with TileContext(nc) as tc:
        with (
            tc.tile_pool(name="consts", bufs=1) as cpool,
            tc.tile_pool(name="xdata", bufs=NS * NG) as xpool,
            tc.tile_pool(name="small", bufs=NS) as spool,
            tc.tile_pool(name="bspl", bufs=1) as bpool,
            tc.tile_pool(name="psum", bufs=2, space="PSUM") as ppool,
        ):
            # ---- sample-0 load doorbells first (starts the big stream),
            # then the small const DMAs, then the remaining loads ----
            xts = {}
            for n in range(NS):
                for g in range(NG):
                    xt = xpool.tile([128, HWPIX], f16, tag="xt")
                    xts[(n, g)] = xt
            for g in range(NG):
                src0 = x_d[0, 128 * g:128 * (g + 1)].rearrange("p h w -> p (h w)")
                nc.sync.dma_start(xts[(0, g)][:], src0)
            w1t_sb = cpool.tile([128, NG * HIDDEN], f16)
            nc.sync.dma_start(w1t_sb[:], w1t_d[:, :])
            sw1_sb = cpool.tile([128, NG * KB * HIDDEN], f16)
            nc.sync.dma_start(sw1_sb[:], sw1_d[:, :])
            w2t_sb = cpool.tile([HIDDEN, C], f16)
            nc.sync.dma_start(w2t_sb[:], w2t_d[:, :])
            sw2_sb = cpool.tile([HIDDEN, KB * C], f16)
            nc.sync.dma_start(sw2_sb[:], sw2_d[:, :])
            gtab_sb = cpool.tile([128, gtab_cols], f32)
            nc.sync.dma_start(gtab_sb[:], gtab_d[:, :])
            # Pre-touch every const tile on VectorE: the DMA-completion wait
            # lands on these throwaway copies, so later DVE consumers (whose
            # ISA formats have a single wait slot) never need a DMA wait.
            touch = cpool.tile([128, 8], f32)
            for i, ct in enumerate((w1t_sb, sw1_sb, gtab_sb)):
                nc.vector.tensor_copy(touch[:, i:i + 1], ct[:, 0:1])
            for i, ct in enumerate((w2t_sb, sw2_sb)):
                nc.vector.tensor_copy(touch[:HIDDEN, 3 + i:4 + i], ct[:, 0:1])
            # Same for TensorE (LDWEIGHTS single wait slot).
            pt_ps = ppool.tile([1, 4], f32, tag="pt", bufs=1)
            for i, ct in enumerate((w1t_sb, sw1_sb)):
                nc.tensor.matmul(pt_ps[0:1, i:i + 1], ct[:, 0:1], ct[:, 0:1],
                                 start=True, stop=True)
            for i, ct in enumerate((w2t_sb, sw2_sb)):
                nc.tensor.matmul(pt_ps[0:1, 2 + i:3 + i], ct[:HIDDEN, 0:1],
                                 ct[:HIDDEN, 0:1], start=True, stop=True)
            # ScalarE: absorb the gtab DMA wait + preload BOTH act tables.
            nc.scalar.activation(touch[:, 5:6], gtab_sb[:, 0:1], Act.Sigmoid)
            nc.scalar.activation(touch[:, 6:7], gtab_sb[:, 0:1], Act.Copy)

            # ---- remaining load doorbells (transfers queue in order) ----
            for n in range(1, NS):
                for g in range(NG):
                    srcn = x_d[n, 128 * g:128 * (g + 1)].rearrange("p h w -> p (h w)")
                    nc.sync.dma_start(xts[(n, g)][:], srcn)

            # samples 0,1 share one sums tile (their KAN runs pair-batched)
            sT01 = spool.tile([128, 2 * NG], f32, tag="sT01", bufs=1)
            sT2 = spool.tile([128, NG], f32, tag="sT", bufs=2)
            sT3 = spool.tile([128, NG], f32, tag="sT", bufs=2)
            # per-tile partial sums (2 chunks per tile, combined by one tiny
            # DVE reduce): a monolithic 4096-elem sum blocks its engine for
            # ~4us and the static scheduler keeps wedging those right in
            # front of the KAN's small critical ops
            pT = cpool.tile([128, 2 * NS * NG], f32)
            scols = {}
            for n in range(NS):
                for g in range(NG):
                    if n < 2:
                        scols[(n, g)] = sT01[:, NG * n + g:NG * n + g + 1]
                    elif n == 2:
                        scols[(n, g)] = sT2[:, g:g + 1]
                    else:
                        scols[(n, g)] = sT3[:, g:g + 1]

            HALF = HWPIX // 2

            def emit_sums(ns):
                for n in ns:
                    for g in range(NG):
                        t = NG * n + g
                        xt = xts[(n, g)]
                        for c in range(2):
                            half = xt[:, HALF * c:HALF * (c + 1)]
                            pcol = pT[:, 2 * t + c:2 * t + c + 1]
                            if SUM_ENG[t] in ("V", "2"):
                                nc.vector.tensor_scalar(
                                    out=half, in0=half, scalar1=1.0,
                                    scalar2=None, op0=Alu.mult, op1=Alu.add,
                                    accum_out=pcol)
                            else:
                                nc.scalar.activation(half, half, Act.Copy,
                                                     accum_out=pcol)
                        nc.vector.reduce_sum(
                            scols[(n, g)], pT[:, 2 * t:2 * t + 2],
                            axis=mybir.AxisListType.X)

            # ---- KAN instances: pair (0,1), then singles 2, 3 ----
            def kan(samples, sT):
                S = len(samples)
                # features bft col = (j*S + s)*NG + g, fp16; j=0 is the silu
                # feature sum*sigmoid(sum/HW) (the 1/HW lives in w1t).
                sig1 = spool.tile([128, S * NG], f32, tag=f"sig1{S}")
                nc.scalar.activation(sig1[:], sT[:, 0:S * NG], Act.Sigmoid,
                                     scale=1.0 / HWPIX)
                bft = spool.tile([128, NF * S * NG], f16, tag=f"bft{S}")
                bft4 = bft.rearrange("p (j s g) -> p j s g", s=S, g=NG)
                sT3 = sT[:, 0:S * NG].rearrange("p (s g) -> p s g", g=NG)
                nc.vector.tensor_tensor(
                    bft4[:, 0], sig1.rearrange("p (s g) -> p s g", g=NG), sT3,
                    Alu.mult)
                out_j = bft.rearrange("p (j s g) -> p s g j", s=S, g=NG)
                # groups 0..2 batched; group 3 alone so only a small emit and
                # 9 matmuls depend on the sample's final tile
                _emit_bsplines(nc, mybir, bpool, gtab_sb, off1,
                               sT3[:, :, 0:3], out_j[:, :, 0:3, 1:NF],
                               128, S, 3, g0=0)
                _emit_bsplines(nc, mybir, bpool, gtab_sb, off1,
                               sT3[:, :, 3:4], out_j[:, :, 3:4, 1:NF],
                               128, S, 1, g0=3)

                # layer-1 matmuls: one accumulation chain over (g, j)
                ps1 = ppool.tile([HIDDEN, S], f32, tag=f"ps1{S}", bufs=(1 if S == 2 else 2))
                nmm = NG * NF
                i = 0
                for g in range(NG):
                    for j in range(NF):
                        if j == 0:
                            lhsT = w1t_sb[:, HIDDEN * g:HIDDEN * (g + 1)]
                        else:
                            col = HIDDEN * (KB * g + (j - 1))
                            lhsT = sw1_sb[:, col:col + HIDDEN]
                        nc.tensor.matmul(
                            ps1[:], lhsT, bft4[:, j, :, g],
                            start=(i == 0), stop=(i == nmm - 1))
                        i += 1

                # inter-layer: t1 = silu(ps1); layer-2 features
                sg = spool.tile([HIDDEN, 2 * S], f32, tag=f"sg2{S}")
                t1 = spool.tile([HIDDEN, S], f32, tag=f"t1{S}")
                nc.scalar.activation(sg[:, 0:S], ps1[:], Act.Sigmoid)
                nc.vector.tensor_tensor(t1[:], sg[:, 0:S], ps1[:], Alu.mult)
                bf2 = spool.tile([HIDDEN, NF * S], f16, tag=f"bf2{S}")
                bf24 = bf2.rearrange("p (j s g) -> p j s g", s=S, g=1)
                nc.scalar.activation(sg[:, S:2 * S], t1[:], Act.Sigmoid)
                nc.vector.tensor_tensor(bf24[:, 0, :, 0], sg[:, S:2 * S], t1[:],
                                        Alu.mult)
                _emit_bsplines(nc, mybir, bpool, gtab_sb, off2,
                               t1.rearrange("p (s g) -> p s g", g=1),
                               bf2.rearrange("p (j s g) -> p s g j", s=S, g=1)
                               [:, :, :, 1:NF],
                               HIDDEN, S, 1)

                # layer-2 per out-group: 9-matmul chain -> sigmoid -> gate
                # multiply -> store doorbell (stores dribble og-by-og)
                for og in range(NG):
                    ps2 = ppool.tile([128, S], f32, tag=f"ps2{S}", bufs=2)
                    for j in range(NF):
                        if j == 0:
                            lhsT = w2t_sb[:, 128 * og:128 * (og + 1)]
                        else:
                            col = C * (j - 1) + 128 * og
                            lhsT = sw2_sb[:, col:col + 128]
                        nc.tensor.matmul(
                            ps2[:], lhsT, bf24[:, j, :, 0],
                            start=(j == 0), stop=(j == NF - 1))
                    gate = spool.tile([128, S], f32, tag=f"gate{S}", bufs=4)
                    nc.scalar.activation(gate[:], ps2[:], Act.Sigmoid)
                    for si, n in enumerate(samples):
                        xt = xts[(n, og)]
                        gcol = gate[:, si:si + 1]
                        if SCALE_ENG[NG * n + og] == "V":
                            nc.vector.tensor_scalar(
                                out=xt[:], in0=xt[:], scalar1=gcol,
                                scalar2=None, op0=Alu.mult)
                        else:
                            nc.scalar.activation(xt[:], xt[:], Act.Copy,
                                                 scale=gcol)
                        dst = y_d[n, 128 * og:128 * (og + 1)].rearrange(
                            "p h w -> p (h w)")
                        nc.sync.dma_start(dst, xt[:])

            emit_sums([0, 1])
            kan([0, 1], sT01)
            emit_sums([2])
            kan([2], sT2)
            emit_sums([3])
            kan([3], sT3)
    nc.compile()
    return nc


def _run(inputs, trace=False):
    from concourse.bass_utils import run_bass_kernel_spmd

    x = np.asarray(inputs["x"])
    assert x.shape == (B, C, H, W), x.shape
    x16 = np.ascontiguousarray(x.astype(np.float16))
    tensors, off1, off2, gtab_cols = _host_prep(inputs)
    nc = _build_nc(off1, off2, gtab_cols)
    in_maps = []
    for c in range(NCORES):
        m = {"x": np.ascontiguousarray(x16[NS * c:NS * (c + 1)])}
        m.update(tensors)
        in_maps.append(m)
    res = run_bass_kernel_spmd(
        nc, in_maps, core_ids=list(range(NCORES)), trace=trace
    )
    out = np.concatenate([res.results[c]["y"] for c in range(NCORES)], axis=0)
    return out.astype(np.float32), res


def kernel(**inputs) -> np.ndarray:
    return _run(inputs)[0]


# revision 15
# speedup vs baseline: 1.0230x; 1.0230x over previous
"""KAN-SE (squeeze-excite with 2-layer KAN MLP) Trainium2 kernel.

Full-input contract: kernel(**inputs) takes the complete (32, 512, 64, 64)
batch plus KAN weights, shards the batch across 8 NeuronCores (4 samples
per core, data-parallel, weights replicated), and returns the full output.

The rel-err gate is 2e-2 (fp32 pipeline measured 4e-7), so precision is
traded for bandwidth/throughput (verified ~4e-4 l2 end to end): x/y move
over HBM as fp16 (host casts both ways), KAN weights/features are fp16 on
the PE, sums/activations stay f32.

v5 pipeline shape (driven by per-instruction traces of v2..v4):
  - const DMAs first (v4 emitted them late and sample-0's KAN stalled
    12us on weights), then all 16 tile-load doorbells, so load transfers
    stream back-to-back at the ~330 GB/s per-core read rate.
  - each load is followed by its row-sum on a rotating engine (DVE
    tensor_scalar+accum / ScalarE Copy+accum, both in-place; the reduce
    path gets no fp16 speedup so it must not serialize on one engine).
    The last tile's sum is a 2-stage DVE op (fp16 4x add of halves, then
    a half-length reduce) to shorten the tail.
  - KAN for samples {0,1} runs pair-batched (2-col rhs); samples 2 and 3
    run alone so their gates chase the load stream.  Layer-1 b-splines
    are emitted groups-0..2 batched + group-3 alone: only ~9 matmuls and
    a small emit depend on a sample's final tile.
  - layer-2 runs per out-group: each 9-matmul chain immediately feeds
    sigmoid -> gate-scale -> store doorbell, so stores dribble out
    og-by-og instead of waiting for the full 512-wide gate.
  - ScalarE only ever runs Sigmoid/Copy (SiLU = x*sigmoid(x), mult on
    DVE); both act tables are preloaded.  Gate scales are mostly DVE
    (fp16 hits the 4x DVE mode, ~1.3us/tile).

Per-core HBM traffic: 16 MiB in + 16 MiB out (fp16), read-once/write-once.
"""

import numpy as np

# ---- problem constants (hardcoded per contract; do not read spec/reference) ----
B, C, H, W = 32, 512, 64, 64
HIDDEN = 64            # max(16, 512 // 8)
KB = 8                 # GRID_SIZE + SPLINE_ORDER = 5 + 3
NCORES = 8
NS = B // NCORES       # samples per core = 4
NG = C // 128          # channel groups of 128 = 4
HWPIX = H * W          # 4096
NF = KB + 1            # features per channel: silu + 8 spline bases

# row-sum engine per tile index t = n*4+g (V=DVE, S=ScalarE, 2=DVE 2-stage);
# ScalarE takes most (3.7us vs DVE 4.4us, and DVE owns the b-splines/scales)
SUM_ENG = "SVSV" "SVSV" "SVSV" "SVS2"
# gate-multiply engine per tile index (DVE 4x fp16 is ~3x faster than ScalarE)
SCALE_ENG = "VSVV" "VVSV" "VVVV" "VVVV"


def _grid_cols(grid_row: np.ndarray, xscale: float, nsg: int):
    """Packed per-group-replicated grid constant columns for the batched
    Cox-de-Boor recurrence, evaluated on inputs x' = x * xscale.

    offsets maps:
      'ge'   -> start of g_i * xscale,        width nsg*12
      (k,'a')-> start of -g_i / (k h),        width nsg*(11-k)
      (k,'c')-> start of  g_{i+k+1} / (k h),  width nsg*(11-k)
      'rs'   -> start of 1/(k h xscale), k=1..3
    """
    g = np.asarray(grid_row, np.float64)
    assert g.shape == (12,)
    h = g[1] - g[0]
    segs, offsets = [], {}
    pos = 0

    def add(key, vals):
        nonlocal pos
        offsets[key] = pos
        segs.append(vals.astype(np.float32))
        pos += vals.size

    add('ge', np.tile(g * xscale, nsg))
    for k in (1, 2, 3):
        w = 11 - k
        add((k, 'a'), np.tile(-g[:w] / (k * h), nsg))
        add((k, 'c'), np.tile(g[k + 1:12] / (k * h), nsg))
    add('rs', np.array([1.0 / (k * h * xscale) for k in (1, 2, 3)]))
    return np.concatenate(segs), offsets


def _host_prep(inputs):
    """Rearrange weights into the SBUF layouts the device program uses."""
    f32, f16 = np.float32, np.float16
    base_w1 = np.asarray(inputs["base_w1"], f32)      # (64, 512)
    spline_w1 = np.asarray(inputs["spline_w1"], f32)  # (64, 512, 8)
    scaler1 = np.asarray(inputs["scaler1"], f32)      # (64, 512)
    base_w2 = np.asarray(inputs["base_w2"], f32)      # (512, 64)
    spline_w2 = np.asarray(inputs["spline_w2"], f32)  # (512, 64, 8)
    scaler2 = np.asarray(inputs["scaler2"], f32)      # (512, 64)

    # layer-1 silu feature arrives as sum*sigmoid(sum/HW) = HW*silu(mean),
    # so fold 1/HW into the base weights.
    # w1t[p, g*64+o] = base_w1[o, 128g+p] / HWPIX
    w1t = (base_w1 / HWPIX).reshape(HIDDEN, NG, 128)
    w1t = w1t.transpose(2, 1, 0).reshape(128, NG * HIDDEN)
    # sw1[p, (g*8+k)*64+o] = (spline_w1*scaler1)[o, 128g+p, k]
    sw1 = (spline_w1 * scaler1[:, :, None]).reshape(HIDDEN, NG, 128, KB)
    sw1 = sw1.transpose(2, 1, 3, 0).reshape(128, NG * KB * HIDDEN)
    # w2t[p, o] = base_w2[o, p]
    w2t = base_w2.T
    # sw2[p, k*512+o] = (spline_w2*scaler2)[o, p, k]
    sw2 = (spline_w2 * scaler2[:, :, None]).transpose(1, 2, 0).reshape(HIDDEN, KB * C)

    # packed grid-constant table: layer1 (on raw sums, xscale=HW, replicated
    # over the 4 groups) then layer2 (xscale=1, single copy)
    c1, off1 = _grid_cols(np.asarray(inputs["grid1"], f32)[0], float(HWPIX), NG)
    c2, off2 = _grid_cols(np.asarray(inputs["grid2"], f32)[0], 1.0, 1)
    off2 = {k: v + c1.size for k, v in off2.items()}
    gtab = np.concatenate([c1, c2])
    gtab_full = np.ascontiguousarray(np.tile(gtab[None, :], (128, 1)))

    tensors = {
        "w1t": np.ascontiguousarray(w1t, f16),
        "sw1": np.ascontiguousarray(sw1, f16),
        "w2t": np.ascontiguousarray(w2t, f16),
        "sw2": np.ascontiguousarray(sw2, f16),
        "gtab": gtab_full,
    }
    return tensors, off1, off2, gtab.size


def _emit_bsplines(nc, mybir, pool, gtab_sb, off, sT3, out_j, P, S, G, g0=0):
    """Cubic B-spline bases for S*G per-partition scalars at once.

    sT3:   AP [P, S, G] of the (pre-scaled) inputs.
    out_j: AP [P, S, G, 8] (may be strided, fp16) for the final bases.
    g0:    first group index (selects the replicated grid-constant cols).
    Grid constants broadcast over S (stride-0); x broadcasts over the basis
    index, so each Cox-de-Boor level is one DVE op over ~S*G*11 elems.
    """
    f32 = mybir.dt.float32
    Alu = mybir.AluOpType

    def rep(key, w):
        o = off[key] + g0 * w
        return gtab_sb[:P, o:o + G * w].rearrange(
            "p (g i) -> p () g i", g=G).broadcast_to([P, S, G, w])

    ge = pool.tile([128, S, G, 12], f32, tag=f"ge{P}{S}{G}", bufs=2)
    xb = sT3.rearrange("p s g -> p s g ()")
    nc.vector.tensor_tensor(
        ge[:P], rep('ge', 12), xb.broadcast_to([P, S, G, 12]), Alu.is_le)
    bprev = pool.tile([128, S, G, 11], f32, tag=f"b0{P}{S}{G}", bufs=2)
    nc.vector.tensor_tensor(
        bprev[:P], ge[:P, :, :, 0:11], ge[:P, :, :, 1:12], Alu.subtract)
    # xr[p, k, s, g] = x * 1/(k h xscale)
    xr = pool.tile([128, 3, S, G], f32, tag=f"xr{P}{S}{G}", bufs=2)
    o = off['rs']
    rs_ap = gtab_sb[:P, o:o + 3].rearrange("p k -> p k () ()")
    nc.vector.tensor_tensor(
        xr[:P], rs_ap.broadcast_to([P, 3, S, G]),
        sT3.rearrange("p s g -> p () s g").broadcast_to([P, 3, S, G]), Alu.mult)
    for k in (1, 2, 3):
        w = 11 - k
        xk = xr[:P, k - 1].rearrange("p s g -> p s g ()").broadcast_to([P, S, G, w])
        a_t = pool.tile([128, S, G, 10], f32, tag=f"bsA{P}{S}{G}", bufs=2)
        c_t = pool.tile([128, S, G, 10], f32, tag=f"bsC{P}{S}{G}", bufs=2)
        # A = (x - g_i)/(k h) = xr + (-g_i/(k h));  C = g_{i+k+1}/(k h) - xr
        nc.vector.tensor_tensor(a_t[:P, :, :, :w], rep((k, 'a'), w), xk, Alu.add)
        nc.vector.tensor_tensor(c_t[:P, :, :, :w], rep((k, 'c'), w), xk, Alu.subtract)
        if k < 3:
            bnext = pool.tile([128, S, G, 10], f32, tag=f"bn{P}{S}{G}", bufs=2)
            outp = bnext[:P, :, :, :w]
        else:
            outp = out_j
        nc.vector.tensor_tensor(
            c_t[:P, :, :, :w], c_t[:P, :, :, :w], bprev[:P, :, :, 1:w + 1], Alu.mult)
        nc.vector.tensor_tensor(outp, a_t[:P, :, :, :w], bprev[:P, :, :, 0:w], Alu.mult)
        nc.vector.tensor_tensor(outp, outp, c_t[:P, :, :, :w], Alu.add)
        if k < 3:
            bprev = bnext


def _build_nc(off1, off2, gtab_cols):
    import concourse.bacc as bacc
    import concourse.bass as bass  # noqa: F401
    import concourse.mybir as mybir
    from concourse.tile import TileContext

    f32 = mybir.dt.float32
    f16 = mybir.dt.float16
    Alu = mybir.AluOpType
    Act = mybir.ActivationFunctionType

    # Bacc (not plain Bass): its compile() runs move_matmul_waits_to_ldweights
    # + generate_event_semaphores, which split multi-waits down to the 1-wait-
    # per-instruction TRN2 ISA limit that walrus enforces.
    nc = bacc.Bacc("TRN2", target_bir_lowering=False)
    x_d = nc.declare_dram_parameter("x", [NS, C, H, W], f16, isOutput=False)
    w1t_d = nc.declare_dram_parameter("w1t", [128, NG * HIDDEN], f16, isOutput=False)
    sw1_d = nc.declare_dram_parameter("sw1", [128, NG * KB * HIDDEN], f16, isOutput=False)
    w2t_d = nc.declare_dram_parameter("w2t", [HIDDEN, C], f16, isOutput=False)
    sw2_d = nc.declare_dram_parameter("sw2", [HIDDEN, KB * C], f16, isOutput=False)
    gtab_d = nc.declare_dram_parameter("gtab", [128, gtab_cols], f32, isOutput=False)
    y_d = nc.declare_dram_parameter("y", [NS, C, H, W], f16, isOutput=True)

    with TileContext(nc) as tc:
        with (
            tc.tile_pool(name="consts", bufs=1) as cpool,
            tc.tile_pool(name="xdata", bufs=NS * NG) as xpool,
            tc.tile_pool(name="small", bufs=NS) as spool,
            tc.tile_pool(name="bspl", bufs=1) as bpool,
            tc.tile_pool(name="psum", bufs=2, space="PSUM") as ppool,
        ):
            # ---- sample-0 load doorbells first (starts the big stream),
            # then the small const DMAs, then the remaining loads ----
            xts = {}
            for n in range(NS):
                for g in range(NG):
                    xt = xpool.tile([128, HWPIX], f16, tag="xt")
                    xts[(n, g)] = xt
            for g in range(NG):
                src0 = x_d[0, 128 * g:128 * (g + 1)].rearrange("p h w -> p (h w)")
                nc.sync.dma_start(xts[(0, g)][:], src0)
            w1t_sb = cpool.tile([128, NG * HIDDEN], f16)
            nc.sync.dma_start(w1t_sb[:], w1t_d[:, :])
            sw1_sb = cpool.tile([128, NG * KB * HIDDEN], f16)
            nc.sync.dma_start(sw1_sb[:], sw1_d[:, :])
            w2t_sb = cpool.tile([HIDDEN, C], f16)
            nc.sync.dma_start(w2t_sb[:], w2t_d[:, :])
            sw2_sb = cpool.tile([HIDDEN, KB * C], f16)
            nc.sync.dma_start(sw2_sb[:], sw2_d[:, :])
            gtab_sb = cpool.tile([128, gtab_cols], f32)
            nc.sync.dma_start(gtab_sb[:], gtab_d[:, :])
            # Pre-touch every const tile on VectorE: the DMA-completion wait
            # lands on these throwaway copies, so later DVE consumers (whose
            # ISA formats have a single wait slot) never need a DMA wait.
            touch = cpool.tile([128, 8], f32)
            for i, ct in enumerate((w1t_sb, sw1_sb, gtab_sb)):
                nc.vector.tensor_copy(touch[:, i:i + 1], ct[:, 0:1])
            for i, ct in enumerate((w2t_sb, sw2_sb)):
                nc.vector.tensor_copy(touch[:HIDDEN, 3 + i:4 + i], ct[:, 0:1])
            # Same for TensorE (LDWEIGHTS single wait slot).
            pt_ps = ppool.tile([1, 4], f32, tag="pt", bufs=1)
            for i, ct in enumerate((w1t_sb, sw1_sb)):
                nc.tensor.matmul(pt_ps[0:1, i:i + 1], ct[:, 0:1], ct[:, 0:1],
                                 start=True, stop=True)
            for i, ct in enumerate((w2t_sb, sw2_sb)):
                nc.tensor.matmul(pt_ps[0:1, 2 + i:3 + i], ct[:HIDDEN, 0:1],
                                 ct[:HIDDEN, 0:1], start=True, stop=True)
            # ScalarE: absorb the gtab DMA wait + preload BOTH act tables.
            nc.scalar.activation(touch[:, 5:6], gtab_sb[:, 0:1], Act.Sigmoid)
            nc.scalar.activation(touch[:, 6:7], gtab_sb[:, 0:1], Act.Copy)

            # ---- remaining load doorbells (transfers queue in order) ----
            for n in range(1, NS):
                for g in range(NG):
                    srcn = x_d[n, 128 * g:128 * (g + 1)].rearrange("p h w -> p (h w)")
                    nc.sync.dma_start(xts[(n, g)][:], srcn)

            # samples 0,1 share one sums tile (their KAN runs pair-batched)
            sT01 = spool.tile([128, 2 * NG], f32, tag="sT01", bufs=1)
            sT2 = spool.tile([128, NG], f32, tag="sT", bufs=2)
            sT3 = spool.tile([128, NG], f32, tag="sT", bufs=2)
            scols = {}
            for n in range(NS):
                for g in range(NG):
                    if n < 2:
                        scols[(n, g)] = sT01[:, NG * n + g:NG * n + g + 1]
                    elif n == 2:
                        scols[(n, g)] = sT2[:, g:g + 1]
                    else:
                        scols[(n, g)] = sT3[:, g:g + 1]

            def emit_sums(ns):
                for n in ns:
                    for g in range(NG):
                        t = NG * n + g
                        xt = xts[(n, g)]
                        scol = scols[(n, g)]
                        eng = SUM_ENG[t]
                        if eng == "V":
                            nc.vector.tensor_scalar(
                                out=xt[:], in0=xt[:], scalar1=1.0, scalar2=None,
                                op0=Alu.mult, op1=Alu.add, accum_out=scol)
                        elif eng == "2":
                            half = bpool.tile([128, HWPIX // 2], f16, tag="half")
                            nc.vector.tensor_tensor(
                                half[:], xt[:, 0:HWPIX // 2], xt[:, HWPIX // 2:],
                                Alu.add)
                            nc.vector.tensor_scalar(
                                out=half[:], in0=half[:], scalar1=1.0,
                                scalar2=None, op0=Alu.mult, op1=Alu.add,
                                accum_out=scol)
                        else:
                            nc.scalar.activation(xt[:], xt[:], Act.Copy,
                                                 accum_out=scol)

            # ---- KAN instances: pair (0,1), then singles 2, 3 ----
            def kan(samples, sT):
                S = len(samples)
                # features bft col = (j*S + s)*NG + g, fp16; j=0 is the silu
                # feature sum*sigmoid(sum/HW) (the 1/HW lives in w1t).
                sig1 = spool.tile([128, S * NG], f32, tag=f"sig1{S}")
                nc.scalar.activation(sig1[:], sT[:, 0:S * NG], Act.Sigmoid,
                                     scale=1.0 / HWPIX)
                bft = spool.tile([128, NF * S * NG], f16, tag=f"bft{S}")
                bft4 = bft.rearrange("p (j s g) -> p j s g", s=S, g=NG)
                sT3 = sT[:, 0:S * NG].rearrange("p (s g) -> p s g", g=NG)
                nc.vector.tensor_tensor(
                    bft4[:, 0], sig1.rearrange("p (s g) -> p s g", g=NG), sT3,
                    Alu.mult)
                out_j = bft.rearrange("p (j s g) -> p s g j", s=S, g=NG)
                # groups 0..2 batched; group 3 alone so only a small emit and
                # 9 matmuls depend on the sample's final tile
                _emit_bsplines(nc, mybir, bpool, gtab_sb, off1,
                               sT3[:, :, 0:3], out_j[:, :, 0:3, 1:NF],
                               128, S, 3, g0=0)
                _emit_bsplines(nc, mybir, bpool, gtab_sb, off1,
                               sT3[:, :, 3:4], out_j[:, :, 3:4, 1:NF],
                               128, S, 1, g0=3)

                # layer-1 matmuls: one accumulation chain over (g, j)
                ps1 = ppool.tile([HIDDEN, S], f32, tag=f"ps1{S}", bufs=(1 if S == 2 else 2))
                nmm = NG * NF
                i = 0
                for g in range(NG):
                    for j in range(NF):
                        if j == 0:
                            lhsT = w1t_sb[:, HIDDEN * g:HIDDEN * (g + 1)]
                        else:
                            col = HIDDEN * (KB * g + (j - 1))
                            lhsT = sw1_sb[:, col:col + HIDDEN]
                        nc.tensor.matmul(
                            ps1[:], lhsT, bft4[:, j, :, g],
                            start=(i == 0), stop=(i == nmm - 1))
                        i += 1

                # inter-layer: t1 = silu(ps1); layer-2 features
                sg = spool.tile([HIDDEN, 2 * S], f32, tag=f"sg2{S}")
                t1 = spool.tile([HIDDEN, S], f32, tag=f"t1{S}")
                nc.scalar.activation(sg[:, 0:S], ps1[:], Act.Sigmoid)
                nc.vector.tensor_tensor(t1[:], sg[:, 0:S], ps1[:], Alu.mult)
                bf2 = spool.tile([HIDDEN, NF * S], f16, tag=f"bf2{S}")
                bf24 = bf2.rearrange("p (j s g) -> p j s g", s=S, g=1)
                nc.scalar.activation(sg[:, S:2 * S], t1[:], Act.Sigmoid)
                nc.vector.tensor_tensor(bf24[:, 0, :, 0], sg[:, S:2 * S], t1[:],
                                        Alu.mult)
                _emit_bsplines(nc, mybir, bpool, gtab_sb, off2,
                               t1.rearrange("p (s g) -> p s g", g=1),
                               bf2.rearrange("p (j s g) -> p s g j", s=S, g=1)
                               [:, :, :, 1:NF],
                               HIDDEN, S, 1)

                # layer-2 per out-group: 9-matmul chain -> sigmoid -> gate
                # multiply -> store doorbell (stores dribble og-by-og)
                for og in range(NG):
                    ps2 = ppool.tile([128, S], f32, tag=f"ps2{S}", bufs=2)
                    for j in range(NF):
                        if j == 0:
                            lhsT = w2t_sb[:, 128 * og:128 * (og + 1)]
                        else:
                            col = C * (j - 1) + 128 * og
                            lhsT = sw2_sb[:, col:col + 128]
                        nc.tensor.matmul(
                            ps2[:], lhsT, bf24[:, j, :, 0],
                            start=(j == 0), stop=(j == NF - 1))
                    gate = spool.tile([128, S], f32, tag=f"gate{S}", bufs=4)
                    nc.scalar.activation(gate[:], ps2[:], Act.Sigmoid)
                    for si, n in enumerate(samples):
                        xt = xts[(n, og)]
                        gcol = gate[:, si:si + 1]
                        if SCALE_ENG[NG * n + og] == "V":
                            nc.vector.tensor_scalar(
                                out=xt[:], in0=xt[:], scalar1=gcol,
                                scalar2=None, op0=Alu.mult)
                        else:
                            nc.scalar.activation(xt[:], xt[:], Act.Copy,
                                                 scale=gcol)
                        dst = y_d[n, 128 * og:128 * (og + 1)].rearrange(
                            "p h w -> p (h w)")
                        nc.sync.dma_start(dst, xt[:])

            emit_sums([0, 1])
            kan([0, 1], sT01)
            # keep s2's big sums from being greedily scheduled into the
            # pair-KAN's serial gaps (in-order engines, no preemption)
            with tc.tile_wait_until(0.000056):
                emit_sums([2])
            kan([2], sT2)
            emit_sums([3])
            kan([3], sT3)
    nc.compile()
    return nc


def _run(inputs, trace=False):
    from concourse.bass_utils import run_bass_kernel_spmd

    x = np.asarray(inputs["x"])
    assert x.shape == (B, C, H, W), x.shape
    x16 = np.ascontiguousarray(x.astype(np.float16))
    tensors, off1, off2, gtab_cols = _host_prep(inputs)
    nc = _build_nc(off1, off2, gtab_cols)
    in_maps = []
    for c in range(NCORES):
        m = {"x": np.ascontiguousarray(x16[NS * c:NS * (c + 1)])}
        m.update(tensors)
        in_maps.append(m)
    res = run_bass_kernel_spmd(
        nc, in_maps, core_ids=list(range(NCORES)), trace=trace
    )
    out = np.concatenate([res.results[c]["y"] for c in range(NCORES)], axis=0)
    return out.astype(np.float32), res


def kernel(**inputs) -> np.ndarray:
    return _run(inputs)[0]
